# revision 14
# baseline (speedup 1.0000x reference)
"""EPMoE (top-2, 16 experts) forward on 8 Trainium2 NeuronCores.

Strategy (expert parallel):
  - Host: router softmax/top-2/renorm, token->expert dispatch (stable order,
    matching the reference), weight re-layout into slab-contiguous form,
    final weighted combine.
  - Device (per core, 2 experts): grouped GEMM1 [Ck,H]x[H,2I] -> silu*up ->
    grouped GEMM2 [Ck,I]x[I,H], all matmuls in f32r (full PE rate), weights
    streamed from HBM as large fully-contiguous slabs (memory-bound
    roofline).

The reference's simulated fp8 quantization (amax scaling + clip, no rounding)
cancels exactly: (x/sa) @ (w/sw)^T * sa*sw == x @ w^T, and the +-448 clip
never binds for amax-scaled values.  So the kernel computes the plain MoE
forward.
"""

import ml_dtypes
import numpy as np

import concourse.bass as bass
import concourse.bacc as bacc
import concourse.mybir as mybir
import concourse.tile as tile
from concourse.bass_utils import run_bass_kernel_spmd

dt = mybir.dt

# Problem shape (hardcoded per spec)
T, H, I, E, TOP_K = 1024, 2048, 1408, 16, 2
TWO_I = 2 * I
N_CORES = 8
EPC = E // N_CORES  # experts per core
CK = 176            # per-expert token capacity on device (>= max expert load)

KT1 = H // 128      # 16 contraction tiles for GEMM1
FT = I // 128       # 11 feature tiles per gate/up half
KT2 = I // 128      # 11 contraction tiles for GEMM2
MT_GRP = 2          # number of GEMM2 m-groups
MT_G = H // 128 // MT_GRP  # 8 output tiles per m-group
MW = MT_G * 128     # 1024, m-group output width
KB1 = 4             # k-tiles per GEMM1 weight slab (one DMA)

# Operand dtype for the matmul data path.  bf16 halves the HBM weight
# traffic (the memory-bound term) and enables FWL on the PE; f32r keeps
# near-fp32 precision at full PE rate but double the DMA bytes.
USE_BF16 = True

_CACHED_NC = None


def _build_program():
    """One SPMD program: per core, 2 experts' MoE FFN over CK padded tokens.

    DRAM layouts are slab-contiguous (host pre-arranged):
      w13t[e, fh, k, p, c] = w13[g, fh*I + c, 128k + p]   (gate/up half fh)
      w2t [e, mg, k2, p, c] = w2[g, mg*MW + c, 128k2 + p]
      xt  [e, p, k, c]     = x[token c of expert g, 128k + p]
      yt  [e, mg, p, m, c] = y^T[h = mg*MW + 128m + p, token c]
    """
    nc = bacc.Bacc("TRN2", target_bir_lowering=False, debug=False,
                   num_devices=N_CORES)

    wdt = dt.bfloat16 if USE_BF16 else dt.float32r
    w13t = nc.declare_dram_parameter("w13t", [EPC, 2, KT1 // KB1, 128, KB1, I],
                                     wdt, isOutput=False)
    w2t = nc.declare_dram_parameter("w2t", [EPC, MT_GRP, 128, KT2, MW],
                                    wdt, isOutput=False)
    xt = nc.declare_dram_parameter("xt", [EPC, 128, KT1, CK], wdt,
                                   isOutput=False)
    yt = nc.declare_dram_parameter("yt", [EPC, MT_GRP, 128, MT_G, CK], dt.float32,
                                   isOutput=True)

    silu_fn = mybir.ActivationFunctionType.Silu

    with tile.TileContext(nc) as tc:
        with (
            tc.tile_pool(name="xpool", bufs=2) as xpool,
            tc.tile_pool(name="w1pool", bufs=5) as w1pool,
            tc.tile_pool(name="w2pool", bufs=3) as w2pool,
            tc.tile_pool(name="spool", bufs=FT + 1) as spool,
            tc.tile_pool(name="apool", bufs=2 * FT) as apool,
            tc.tile_pool(name="ypool", bufs=2) as ypool,
            tc.tile_pool(name="psum", bufs=8, space="PSUM") as pspool,
        ):
            for e in range(EPC):
                # resident dispatched activations, transposed: [128, k, Ck]
                xte = xpool.tile([128, KT1, CK], wdt, tag="xte")
                # SWDGE keeps the HWDGE weight-stream queues free of
                # head-of-line waits from I/O transfers.
                nc.gpsimd.dma_start(xte[:], xt[e, :, :, :])

                # ---- GEMM1 + silu_and_mul ----
                silu_tiles = []
                act_tiles = []
                for fh in range(2):  # 0 = gate half, 1 = up half
                    pst = [pspool.tile([128, 2 * CK], dt.float32, tag="ps",
                                       name=f"ps1_{e}_{fh}_{i}")
                           for i in range((FT + 1) // 2)]
                    for kh in range(KT1 // KB1):
                        slab = w1pool.tile([128, KB1, I], wdt, tag="w13")
                        # Sub-slab DMA pieces: matmuls start on partial
                        # slabs (subtile deps), smoothing slab-edge stalls;
                        # finest pieces on the very first phase so the PE
                        # pipeline fills as early as possible.
                        np_pieces = KB1 if (e == 0 and fh == 0 and kh < 2) else 2
                        step = KB1 // np_pieces
                        for pi in range(np_pieces):
                            eng = nc.sync if (kh * np_pieces + pi) % 2 == 0 else nc.scalar
                            lo = pi * step
                            eng.dma_start(slab[:, lo:lo + step, :],
                                          w13t[e, fh, kh, :, lo:lo + step, :])
                        for kk in range(KB1):
                            k = kh * KB1 + kk
                            for j in range(FT):
                                dst = pst[j // 2][:,
                                                  (j % 2) * CK:(j % 2 + 1) * CK]
                                # start=True clears has_written for the WHOLE
                                # bank: only the first group packed into each
                                # bank may set it.  The second group's k==0
                                # matmul overwrites (its bits are clear).
                                nc.tensor.matmul(
                                    dst,
                                    slab[:, kk, j * 128:(j + 1) * 128],
                                    xte[:, k, :],
                                    start=(k == 0 and j % 2 == 0),
                                    stop=(k == KT1 - 1),
                                    skip_group_check=(j % 2 == 1),
                                )
                    for j in range(FT):
                        src = pst[j // 2][:, (j % 2) * CK:(j % 2 + 1) * CK]
                        if fh == 0:
                            st = spool.tile([128, CK], wdt, tag="silu",
                                            name=f"silu_{e}_{j}")
                            nc.scalar.activation(st[:], src, silu_fn)
                            silu_tiles.append(st)
                        else:
                            at = apool.tile([128, CK], wdt, tag="act",
                                            name=f"act_{e}_{j}")
                            nc.vector.tensor_mul(at[:], silu_tiles[j][:], src)
                            act_tiles.append(at)

                # ---- GEMM2 ----
                for mg in range(MT_GRP):
                    pst2 = [pspool.tile([128, 2 * CK], dt.float32, tag="ps",
                                        name=f"ps2_{e}_{mg}_{i}")
                            for i in range(MT_G // 2)]
                    slab2 = w2pool.tile([128, KT2, MW], wdt, tag="w2")
                    for pi, (lo, hi) in enumerate(((0, 4), (4, 8), (8, KT2))):
                        eng = nc.sync if (mg + pi) % 2 == 0 else nc.scalar
                        eng.dma_start(slab2[:, lo:hi, :],
                                      w2t[e, mg, :, lo:hi, :])
                    for k2 in range(KT2):
                        for m in range(MT_G):
                            dst = pst2[m // 2][:, (m % 2) * CK:(m % 2 + 1) * CK]
                            nc.tensor.matmul(
                                dst,
                                slab2[:, k2, m * 128:(m + 1) * 128],
                                act_tiles[k2][:],
                                start=(k2 == 0 and m % 2 == 0),
                                stop=(k2 == KT2 - 1),
                                skip_group_check=(m % 2 == 1),
                            )
                    ybig = ypool.tile([128, MT_G, CK], dt.float32, tag="y")
                    for m in range(MT_G):
                        src = pst2[m // 2][:, (m % 2) * CK:(m % 2 + 1) * CK]
                        nc.vector.tensor_copy(ybig[:, m, :], src)
                    nc.gpsimd.dma_start(yt[e, mg, :, :, :], ybig[:])

    nc.compile()
    return nc


def _get_program():
    global _CACHED_NC
    if _CACHED_NC is None:
        _CACHED_NC = _build_program()
    return _CACHED_NC


def _route(router_logits):
    """Replicate the reference routing in numpy (fp32)."""
    lm = router_logits - router_logits.max(axis=-1, keepdims=True)
    p = np.exp(lm)
    probs = p / p.sum(axis=-1, keepdims=True)
    topi = np.argsort(-probs, axis=-1, kind="stable")[:, :TOP_K]
    topw = np.take_along_axis(probs, topi, axis=-1)
    topw = topw / topw.sum(axis=-1, keepdims=True)

    rid = topi.reshape(-1)
    rtok = np.arange(T * TOP_K) // TOP_K
    order = np.argsort(rid, kind="stable")
    counts = np.bincount(rid, minlength=E)
    offsets = np.concatenate([[0], np.cumsum(counts)[:-1]])
    return topw, rid, rtok, order, counts, offsets


def _np_wdt():
    return ml_dtypes.bfloat16 if USE_BF16 else np.float32


def _prep_weights(w13_weight, w2_weight):
    """Slab-contiguous per-core weight shards (see _build_program)."""
    w13t_cores, w2t_cores = [], []
    for c in range(N_CORES):
        a = np.empty((EPC, 2, KT1 // KB1, 128, KB1, I), _np_wdt())
        b = np.empty((EPC, MT_GRP, 128, KT2, MW), _np_wdt())
        for el in range(EPC):
            g = c * EPC + el
            # [H, 2I] -> (kh, kk, p, fh, c) -> (fh, kh, p, kk, c)
            a[el] = (
                w13_weight[g].T
                .reshape(KT1 // KB1, KB1, 128, 2, I)
                .transpose(3, 0, 2, 1, 4)
            )
            # [I, H] -> (k2, p, mg, c) -> (mg, p, k2, c)
            b[el] = (
                w2_weight[g].T.reshape(KT2, 128, MT_GRP, MW)
                .transpose(2, 1, 0, 3)
            )
        w13t_cores.append(a)
        w2t_cores.append(b)
    return w13t_cores, w2t_cores


def kernel(x, router_logits, w13_weight, w2_weight):
    x = np.asarray(x, dtype=np.float32)
    router_logits = np.asarray(router_logits, dtype=np.float32)
    w13_weight = np.asarray(w13_weight, dtype=np.float32)
    w2_weight = np.asarray(w2_weight, dtype=np.float32)
    assert x.shape == (T, H) and router_logits.shape == (T, E)
    assert w13_weight.shape == (E, TWO_I, H) and w2_weight.shape == (E, H, I)

    topw, rid, rtok, order, counts, offsets = _route(router_logits)
    n_chunks = max(1, -(-int(counts.max()) // CK))  # ceil; 1 in practice

    nc = _get_program()
    w13t_cores, w2t_cores = _prep_weights(w13_weight, w2_weight)

    # token rows per expert, in reference (stable) dispatch order
    expert_rows = [
        order[offsets[g]:offsets[g] + counts[g]] for g in range(E)
    ]

    ybuf = np.zeros((E, int(counts.max()), H), np.float32)

    for chunk in range(n_chunks):
        in_maps = []
        for c in range(N_CORES):
            xt_c = np.zeros((EPC, 128, KT1, CK), _np_wdt())
            for el in range(EPC):
                g = c * EPC + el
                rows = expert_rows[g][chunk * CK:(chunk + 1) * CK]
                if len(rows):
                    # [n, H] -> [H, n] -> [KT1, 128, n] -> [128, KT1, n]
                    xt_c[el, :, :, :len(rows)] = (
                        x[rtok[rows]].T.reshape(KT1, 128, -1).transpose(1, 0, 2)
                    )
            in_maps.append(
                {"w13t": w13t_cores[c], "w2t": w2t_cores[c], "xt": xt_c}
            )
        res = run_bass_kernel_spmd(nc, in_maps, list(range(N_CORES)))
        for c in range(N_CORES):
            yt_c = res.results[c]["yt"]  # [EPC, 2, 128, MT_G, CK]
            for el in range(EPC):
                g = c * EPC + el
                n = len(expert_rows[g][chunk * CK:(chunk + 1) * CK])
                if n:
                    lo = chunk * CK
                    # [MT_GRP, 128, MT_G, CK] -> y^T [H, CK] -> [n, H]
                    ytr = (
                        yt_c[el].transpose(0, 2, 1, 3).reshape(H, CK)
                    )
                    ybuf[g, lo:lo + n] = ytr[:, :n].T

    # ---- combine: gather rows back, weight by router probs ----
    pos = np.empty(T * TOP_K, np.int64)
    for g in range(E):
        pos[expert_rows[g]] = np.arange(counts[g])
    yrows = ybuf[rid, pos]  # [T*K, H]
    out = np.einsum(
        "tkh,tk->th", yrows.reshape(T, TOP_K, H), topw.astype(np.float32)
    )
    return out.astype(np.float32)


# revision 15
# speedup vs baseline: 1.0373x; 1.0373x over previous
"""EPMoE (top-2, 16 experts) forward on 8 Trainium2 NeuronCores.

Strategy (expert parallel):
  - Host: router softmax/top-2/renorm, token->expert dispatch (stable order,
    matching the reference), weight re-layout into slab-contiguous form,
    final weighted combine.
  - Device (per core, 2 experts): grouped GEMM1 [Ck,H]x[H,2I] -> silu*up ->
    grouped GEMM2 [Ck,I]x[I,H], all matmuls in f32r (full PE rate), weights
    streamed from HBM as large fully-contiguous slabs (memory-bound
    roofline).

The reference's simulated fp8 quantization (amax scaling + clip, no rounding)
cancels exactly: (x/sa) @ (w/sw)^T * sa*sw == x @ w^T, and the +-448 clip
never binds for amax-scaled values.  So the kernel computes the plain MoE
forward.
"""

import ml_dtypes
import numpy as np

import concourse.bass as bass
import concourse.bacc as bacc
import concourse.mybir as mybir
import concourse.tile as tile
from concourse.bass_utils import run_bass_kernel_spmd

dt = mybir.dt

# Problem shape (hardcoded per spec)
T, H, I, E, TOP_K = 1024, 2048, 1408, 16, 2
TWO_I = 2 * I
N_CORES = 8
EPC = E // N_CORES  # experts per core
CK = 176            # per-expert token capacity on device (>= max expert load)

KT1 = H // 128      # 16 contraction tiles for GEMM1
FT = I // 128       # 11 feature tiles per gate/up half
KT2 = I // 128      # 11 contraction tiles for GEMM2
MT_GRP = 2          # number of GEMM2 m-groups
MT_G = H // 128 // MT_GRP  # 8 output tiles per m-group
MW = MT_G * 128     # 1024, m-group output width
KB1 = 4             # k-tiles per GEMM1 weight slab (one DMA)

# Operand dtype for the matmul data path.  bf16 halves the HBM weight
# traffic (the memory-bound term) and enables FWL on the PE; f32r keeps
# near-fp32 precision at full PE rate but double the DMA bytes.
USE_BF16 = True

_CACHED_NC = None


def _build_program():
    """One SPMD program: per core, 2 experts' MoE FFN over CK padded tokens.

    DRAM layouts are slab-contiguous (host pre-arranged):
      w13t[e, fh, k, p, c] = w13[g, fh*I + c, 128k + p]   (gate/up half fh)
      w2t [e, mg, k2, p, c] = w2[g, mg*MW + c, 128k2 + p]
      xt  [e, p, k, c]     = x[token c of expert g, 128k + p]
      yt  [e, mg, p, m, c] = y^T[h = mg*MW + 128m + p, token c]
    """
    nc = bacc.Bacc("TRN2", target_bir_lowering=False, debug=False,
                   num_devices=N_CORES)

    wdt = dt.bfloat16 if USE_BF16 else dt.float32r
    w13t = nc.declare_dram_parameter("w13t", [EPC, 2, KT1 // KB1, 128, KB1, I],
                                     wdt, isOutput=False)
    w2t = nc.declare_dram_parameter("w2t", [EPC, MT_GRP, 128, KT2, MW],
                                    wdt, isOutput=False)
    xt = nc.declare_dram_parameter("xt", [EPC, 128, KT1, CK], wdt,
                                   isOutput=False)
    yt = nc.declare_dram_parameter("yt", [EPC, MT_GRP, 128, MT_G, CK], dt.float32,
                                   isOutput=True)

    silu_fn = mybir.ActivationFunctionType.Silu

    with tile.TileContext(nc) as tc:
        with (
            tc.tile_pool(name="xpool", bufs=2) as xpool,
            tc.tile_pool(name="w1pool", bufs=5) as w1pool,
            tc.tile_pool(name="w2pool", bufs=3) as w2pool,
            tc.tile_pool(name="spool", bufs=FT + 1) as spool,
            tc.tile_pool(name="apool", bufs=2 * FT) as apool,
            tc.tile_pool(name="ypool", bufs=2) as ypool,
            tc.tile_pool(name="psum", bufs=8, space="PSUM") as pspool,
        ):
            for e in range(EPC):
                # resident dispatched activations, transposed: [128, k, Ck]
                xte = xpool.tile([128, KT1, CK], wdt, tag="xte")
                # SWDGE keeps the HWDGE weight-stream queues free of
                # head-of-line waits from I/O transfers.
                nc.gpsimd.dma_start(xte[:], xt[e, :, :, :])

                # ---- GEMM1 + silu_and_mul ----
                silu_tiles = []
                act_tiles = []
                for fh in range(2):  # 0 = gate half, 1 = up half
                    pst = [pspool.tile([128, 2 * CK], dt.float32, tag="ps",
                                       name=f"ps1_{e}_{fh}_{i}")
                           for i in range((FT + 1) // 2)]
                    for kh in range(KT1 // KB1):
                        slab = w1pool.tile([128, KB1, I], wdt, tag="w13")
                        # Sub-slab DMA pieces: matmuls start on partial
                        # slabs (subtile deps), smoothing slab-edge stalls;
                        # finest pieces on the very first phase so the PE
                        # pipeline fills as early as possible.
                        np_pieces = KB1 if (e == 0 and fh == 0 and kh < 2) else 1
                        step = KB1 // np_pieces
                        for pi in range(np_pieces):
                            eng = nc.sync if (kh * np_pieces + pi) % 2 == 0 else nc.scalar
                            lo = pi * step
                            eng.dma_start(slab[:, lo:lo + step, :],
                                          w13t[e, fh, kh, :, lo:lo + step, :])
                        for kk in range(KB1):
                            k = kh * KB1 + kk
                            for j in range(FT):
                                dst = pst[j // 2][:,
                                                  (j % 2) * CK:(j % 2 + 1) * CK]
                                # start=True clears has_written for the WHOLE
                                # bank: only the first group packed into each
                                # bank may set it.  The second group's k==0
                                # matmul overwrites (its bits are clear).
                                nc.tensor.matmul(
                                    dst,
                                    slab[:, kk, j * 128:(j + 1) * 128],
                                    xte[:, k, :],
                                    start=(k == 0 and j % 2 == 0),
                                    stop=(k == KT1 - 1),
                                    skip_group_check=(j % 2 == 1),
                                )
                    for j in range(FT):
                        src = pst[j // 2][:, (j % 2) * CK:(j % 2 + 1) * CK]
                        if fh == 0:
                            st = spool.tile([128, CK], wdt, tag="silu",
                                            name=f"silu_{e}_{j}")
                            nc.scalar.activation(st[:], src, silu_fn)
                            silu_tiles.append(st)
                        else:
                            at = apool.tile([128, CK], wdt, tag="act",
                                            name=f"act_{e}_{j}")
                            nc.vector.tensor_mul(at[:], silu_tiles[j][:], src)
                            act_tiles.append(at)

                # ---- GEMM2 ----
                for mg in range(MT_GRP):
                    pst2 = [pspool.tile([128, 2 * CK], dt.float32, tag="ps",
                                        name=f"ps2_{e}_{mg}_{i}")
                            for i in range(MT_G // 2)]
                    slab2 = w2pool.tile([128, KT2, MW], wdt, tag="w2")
                    for pi, (lo, hi) in enumerate(((0, 4), (4, 8), (8, KT2))):
                        eng = nc.sync if (mg + pi) % 2 == 0 else nc.scalar
                        eng.dma_start(slab2[:, lo:hi, :],
                                      w2t[e, mg, :, lo:hi, :])
                    for k2 in range(KT2):
                        for m in range(MT_G):
                            dst = pst2[m // 2][:, (m % 2) * CK:(m % 2 + 1) * CK]
                            nc.tensor.matmul(
                                dst,
                                slab2[:, k2, m * 128:(m + 1) * 128],
                                act_tiles[k2][:],
                                start=(k2 == 0 and m % 2 == 0),
                                stop=(k2 == KT2 - 1),
                                skip_group_check=(m % 2 == 1),
                            )
                    ybig = ypool.tile([128, MT_G, CK], dt.float32, tag="y")
                    for m in range(MT_G):
                        src = pst2[m // 2][:, (m % 2) * CK:(m % 2 + 1) * CK]
                        nc.vector.tensor_copy(ybig[:, m, :], src)
                    if e == EPC - 1 and mg == MT_GRP - 1:
                        nc.sync.dma_start(yt[e, mg, :, :, :], ybig[:])
                    else:
                        nc.gpsimd.dma_start(yt[e, mg, :, :, :], ybig[:])

    nc.compile()
    return nc


def _get_program():
    global _CACHED_NC
    if _CACHED_NC is None:
        _CACHED_NC = _build_program()
    return _CACHED_NC


def _route(router_logits):
    """Replicate the reference routing in numpy (fp32)."""
    lm = router_logits - router_logits.max(axis=-1, keepdims=True)
    p = np.exp(lm)
    probs = p / p.sum(axis=-1, keepdims=True)
    topi = np.argsort(-probs, axis=-1, kind="stable")[:, :TOP_K]
    topw = np.take_along_axis(probs, topi, axis=-1)
    topw = topw / topw.sum(axis=-1, keepdims=True)

    rid = topi.reshape(-1)
    rtok = np.arange(T * TOP_K) // TOP_K
    order = np.argsort(rid, kind="stable")
    counts = np.bincount(rid, minlength=E)
    offsets = np.concatenate([[0], np.cumsum(counts)[:-1]])
    return topw, rid, rtok, order, counts, offsets


def _np_wdt():
    return ml_dtypes.bfloat16 if USE_BF16 else np.float32


def _prep_weights(w13_weight, w2_weight):
    """Slab-contiguous per-core weight shards (see _build_program)."""
    w13t_cores, w2t_cores = [], []
    for c in range(N_CORES):
        a = np.empty((EPC, 2, KT1 // KB1, 128, KB1, I), _np_wdt())
        b = np.empty((EPC, MT_GRP, 128, KT2, MW), _np_wdt())
        for el in range(EPC):
            g = c * EPC + el
            # [H, 2I] -> (kh, kk, p, fh, c) -> (fh, kh, p, kk, c)
            a[el] = (
                w13_weight[g].T
                .reshape(KT1 // KB1, KB1, 128, 2, I)
                .transpose(3, 0, 2, 1, 4)
            )
            # [I, H] -> (k2, p, mg, c) -> (mg, p, k2, c)
            b[el] = (
                w2_weight[g].T.reshape(KT2, 128, MT_GRP, MW)
                .transpose(2, 1, 0, 3)
            )
        w13t_cores.append(a)
        w2t_cores.append(b)
    return w13t_cores, w2t_cores


def kernel(x, router_logits, w13_weight, w2_weight):
    x = np.asarray(x, dtype=np.float32)
    router_logits = np.asarray(router_logits, dtype=np.float32)
    w13_weight = np.asarray(w13_weight, dtype=np.float32)
    w2_weight = np.asarray(w2_weight, dtype=np.float32)
    assert x.shape == (T, H) and router_logits.shape == (T, E)
    assert w13_weight.shape == (E, TWO_I, H) and w2_weight.shape == (E, H, I)

    topw, rid, rtok, order, counts, offsets = _route(router_logits)
    n_chunks = max(1, -(-int(counts.max()) // CK))  # ceil; 1 in practice

    nc = _get_program()
    w13t_cores, w2t_cores = _prep_weights(w13_weight, w2_weight)

    # token rows per expert, in reference (stable) dispatch order
    expert_rows = [
        order[offsets[g]:offsets[g] + counts[g]] for g in range(E)
    ]

    ybuf = np.zeros((E, int(counts.max()), H), np.float32)

    for chunk in range(n_chunks):
        in_maps = []
        for c in range(N_CORES):
            xt_c = np.zeros((EPC, 128, KT1, CK), _np_wdt())
            for el in range(EPC):
                g = c * EPC + el
                rows = expert_rows[g][chunk * CK:(chunk + 1) * CK]
                if len(rows):
                    # [n, H] -> [H, n] -> [KT1, 128, n] -> [128, KT1, n]
                    xt_c[el, :, :, :len(rows)] = (
                        x[rtok[rows]].T.reshape(KT1, 128, -1).transpose(1, 0, 2)
                    )
            in_maps.append(
                {"w13t": w13t_cores[c], "w2t": w2t_cores[c], "xt": xt_c}
            )
        res = run_bass_kernel_spmd(nc, in_maps, list(range(N_CORES)))
        for c in range(N_CORES):
            yt_c = res.results[c]["yt"]  # [EPC, 2, 128, MT_G, CK]
            for el in range(EPC):
                g = c * EPC + el
                n = len(expert_rows[g][chunk * CK:(chunk + 1) * CK])
                if n:
                    lo = chunk * CK
                    # [MT_GRP, 128, MT_G, CK] -> y^T [H, CK] -> [n, H]
                    ytr = (
                        yt_c[el].transpose(0, 2, 1, 3).reshape(H, CK)
                    )
                    ybuf[g, lo:lo + n] = ytr[:, :n].T

    # ---- combine: gather rows back, weight by router probs ----
    pos = np.empty(T * TOP_K, np.int64)
    for g in range(E):
        pos[expert_rows[g]] = np.arange(counts[g])
    yrows = ybuf[rid, pos]  # [T*K, H]
    out = np.einsum(
        "tkh,tk->th", yrows.reshape(T, TOP_K, H), topw.astype(np.float32)
    )
    return out.astype(np.float32)


# revision 16
# speedup vs baseline: 1.1106x; 1.0706x over previous
"""EPMoE (top-2, 16 experts) forward on 8 Trainium2 NeuronCores.

Strategy (expert parallel):
  - Host: router softmax/top-2/renorm, token->expert dispatch (stable order,
    matching the reference), weight re-layout into slab-contiguous form,
    final weighted combine.
  - Device (per core, 2 experts): grouped GEMM1 [Ck,H]x[H,2I] -> silu*up ->
    grouped GEMM2 [Ck,I]x[I,H], all matmuls in f32r (full PE rate), weights
    streamed from HBM as large fully-contiguous slabs (memory-bound
    roofline).

The reference's simulated fp8 quantization (amax scaling + clip, no rounding)
cancels exactly: (x/sa) @ (w/sw)^T * sa*sw == x @ w^T, and the +-448 clip
never binds for amax-scaled values.  So the kernel computes the plain MoE
forward.
"""

import ml_dtypes
import numpy as np

import concourse.bass as bass
import concourse.bacc as bacc
import concourse.mybir as mybir
import concourse.tile as tile
from concourse.bass_utils import run_bass_kernel_spmd

dt = mybir.dt

# Problem shape (hardcoded per spec)
T, H, I, E, TOP_K = 1024, 2048, 1408, 16, 2
TWO_I = 2 * I
N_CORES = 8
EPC = E // N_CORES  # experts per core
CK = 176            # per-expert token capacity on device (>= max expert load)

KT1 = H // 128      # 16 contraction tiles for GEMM1
FT = I // 128       # 11 feature tiles per gate/up half
KT2 = I // 128      # 11 contraction tiles for GEMM2
MT_GRP = 2          # number of GEMM2 m-groups
MT_G = H // 128 // MT_GRP  # 8 output tiles per m-group
MW = MT_G * 128     # 1024, m-group output width
KB1 = 4             # k-tiles per GEMM1 weight slab (one DMA)

# Operand dtype for the matmul data path.  bf16 halves the HBM weight
# traffic (the memory-bound term) and enables FWL on the PE; f32r keeps
# near-fp32 precision at full PE rate but double the DMA bytes.
USE_BF16 = True

_CACHED_NC = None


def _build_program():
    """One SPMD program: per core, 2 experts' MoE FFN over CK padded tokens.

    DRAM layouts are slab-contiguous (host pre-arranged):
      w13t[e, fh, k, p, c] = w13[g, fh*I + c, 128k + p]   (gate/up half fh)
      w2t [e, mg, k2, p, c] = w2[g, mg*MW + c, 128k2 + p]
      xt  [e, p, k, c]     = x[token c of expert g, 128k + p]
      yt  [e, mg, p, m, c] = y^T[h = mg*MW + 128m + p, token c]
    """
    nc = bacc.Bacc("TRN2", target_bir_lowering=False, debug=False,
                   num_devices=N_CORES)

    wdt = dt.bfloat16 if USE_BF16 else dt.float32r
    w13t = nc.declare_dram_parameter("w13t", [EPC, 2, KT1 // KB1, 128, KB1, I],
                                     wdt, isOutput=False)
    w2t = nc.declare_dram_parameter("w2t", [EPC, MT_GRP, 128, KT2, MW],
                                    wdt, isOutput=False)
    xt = nc.declare_dram_parameter("xt", [EPC, 128, KT1, CK], wdt,
                                   isOutput=False)
    yt = nc.declare_dram_parameter("yt", [EPC, MT_GRP, 128, MT_G, CK], dt.float32,
                                   isOutput=True)

    silu_fn = mybir.ActivationFunctionType.Silu

    with tile.TileContext(nc) as tc:
        with (
            tc.tile_pool(name="xpool", bufs=2) as xpool,
            tc.tile_pool(name="w1pool", bufs=5) as w1pool,
            tc.tile_pool(name="w2pool", bufs=3) as w2pool,
            tc.tile_pool(name="spool", bufs=FT + 1) as spool,
            tc.tile_pool(name="apool", bufs=2 * FT) as apool,
            tc.tile_pool(name="ypool", bufs=2) as ypool,
            tc.tile_pool(name="psum", bufs=8, space="PSUM") as pspool,
        ):
            for e in range(EPC):
                # resident dispatched activations, transposed: [128, k, Ck]
                xte = xpool.tile([128, KT1, CK], wdt, tag="xte")
                # SWDGE keeps the HWDGE weight-stream queues free of
                # head-of-line waits from I/O transfers.
                nc.gpsimd.dma_start(xte[:], xt[e, :, :, :])

                # ---- GEMM1 + silu_and_mul ----
                silu_tiles = []
                act_tiles = []
                for fh in range(2):  # 0 = gate half, 1 = up half
                    pst = [pspool.tile([128, 2 * CK], dt.float32, tag="ps",
                                       name=f"ps1_{e}_{fh}_{i}")
                           for i in range((FT + 1) // 2)]
                    for kh in range(KT1 // KB1):
                        slab = w1pool.tile([128, KB1, I], wdt, tag="w13")
                        # Sub-slab DMA pieces: matmuls start on partial
                        # slabs (subtile deps), smoothing slab-edge stalls;
                        # finest pieces on the very first phase so the PE
                        # pipeline fills as early as possible.
                        np_pieces = KB1 if (e == 0 and fh == 0 and kh < 2) else 2
                        step = KB1 // np_pieces
                        for pi in range(np_pieces):
                            eng = nc.sync if (kh * np_pieces + pi) % 2 == 0 else nc.scalar
                            lo = pi * step
                            eng.dma_start(slab[:, lo:lo + step, :],
                                          w13t[e, fh, kh, :, lo:lo + step, :])
                        for kk in range(KB1):
                            k = kh * KB1 + kk
                            for j in range(FT):
                                dst = pst[j // 2][:,
                                                  (j % 2) * CK:(j % 2 + 1) * CK]
                                # start=True clears has_written for the WHOLE
                                # bank: only the first group packed into each
                                # bank may set it.  The second group's k==0
                                # matmul overwrites (its bits are clear).
                                nc.tensor.matmul(
                                    dst,
                                    slab[:, kk, j * 128:(j + 1) * 128],
                                    xte[:, k, :],
                                    start=(k == 0 and j % 2 == 0),
                                    stop=(k == KT1 - 1),
                                    skip_group_check=(j % 2 == 1),
                                )
                    for j in range(FT):
                        src = pst[j // 2][:, (j % 2) * CK:(j % 2 + 1) * CK]
                        if fh == 0:
                            st = spool.tile([128, CK], wdt, tag="silu",
                                            name=f"silu_{e}_{j}")
                            nc.scalar.activation(st[:], src, silu_fn)
                            silu_tiles.append(st)
                        else:
                            at = apool.tile([128, CK], wdt, tag="act",
                                            name=f"act_{e}_{j}")
                            nc.vector.tensor_mul(at[:], silu_tiles[j][:], src)
                            act_tiles.append(at)

                # ---- GEMM2 ----
                for mg in range(MT_GRP):
                    pst2 = [pspool.tile([128, 2 * CK], dt.float32, tag="ps",
                                        name=f"ps2_{e}_{mg}_{i}")
                            for i in range(MT_G // 2)]
                    slab2 = w2pool.tile([128, KT2, MW], wdt, tag="w2")
                    for pi, (lo, hi) in enumerate(((0, 4), (4, 8), (8, KT2))):
                        eng = nc.sync if (mg + pi) % 2 == 0 else nc.scalar
                        eng.dma_start(slab2[:, lo:hi, :],
                                      w2t[e, mg, :, lo:hi, :])
                    for k2 in range(KT2):
                        for m in range(MT_G):
                            dst = pst2[m // 2][:, (m % 2) * CK:(m % 2 + 1) * CK]
                            nc.tensor.matmul(
                                dst,
                                slab2[:, k2, m * 128:(m + 1) * 128],
                                act_tiles[k2][:],
                                start=(k2 == 0 and m % 2 == 0),
                                stop=(k2 == KT2 - 1),
                                skip_group_check=(m % 2 == 1),
                            )
                    ybig = ypool.tile([128, MT_G, CK], dt.float32, tag="y")
                    for m in range(MT_G):
                        src = pst2[m // 2][:, (m % 2) * CK:(m % 2 + 1) * CK]
                        nc.vector.tensor_copy(ybig[:, m, :], src)
                    if e == EPC - 1 and mg == MT_GRP - 1:
                        nc.sync.dma_start(yt[e, mg, :, :, :], ybig[:])
                    else:
                        nc.gpsimd.dma_start(yt[e, mg, :, :, :], ybig[:])

    nc.compile()
    return nc


def _get_program():
    global _CACHED_NC
    if _CACHED_NC is None:
        _CACHED_NC = _build_program()
    return _CACHED_NC


def _route(router_logits):
    """Replicate the reference routing in numpy (fp32)."""
    lm = router_logits - router_logits.max(axis=-1, keepdims=True)
    p = np.exp(lm)
    probs = p / p.sum(axis=-1, keepdims=True)
    topi = np.argsort(-probs, axis=-1, kind="stable")[:, :TOP_K]
    topw = np.take_along_axis(probs, topi, axis=-1)
    topw = topw / topw.sum(axis=-1, keepdims=True)

    rid = topi.reshape(-1)
    rtok = np.arange(T * TOP_K) // TOP_K
    order = np.argsort(rid, kind="stable")
    counts = np.bincount(rid, minlength=E)
    offsets = np.concatenate([[0], np.cumsum(counts)[:-1]])
    return topw, rid, rtok, order, counts, offsets


def _np_wdt():
    return ml_dtypes.bfloat16 if USE_BF16 else np.float32


def _prep_weights(w13_weight, w2_weight):
    """Slab-contiguous per-core weight shards (see _build_program)."""
    w13t_cores, w2t_cores = [], []
    for c in range(N_CORES):
        a = np.empty((EPC, 2, KT1 // KB1, 128, KB1, I), _np_wdt())
        b = np.empty((EPC, MT_GRP, 128, KT2, MW), _np_wdt())
        for el in range(EPC):
            g = c * EPC + el
            # [H, 2I] -> (kh, kk, p, fh, c) -> (fh, kh, p, kk, c)
            a[el] = (
                w13_weight[g].T
                .reshape(KT1 // KB1, KB1, 128, 2, I)
                .transpose(3, 0, 2, 1, 4)
            )
            # [I, H] -> (k2, p, mg, c) -> (mg, p, k2, c)
            b[el] = (
                w2_weight[g].T.reshape(KT2, 128, MT_GRP, MW)
                .transpose(2, 1, 0, 3)
            )
        w13t_cores.append(a)
        w2t_cores.append(b)
    return w13t_cores, w2t_cores


def kernel(x, router_logits, w13_weight, w2_weight):
    x = np.asarray(x, dtype=np.float32)
    router_logits = np.asarray(router_logits, dtype=np.float32)
    w13_weight = np.asarray(w13_weight, dtype=np.float32)
    w2_weight = np.asarray(w2_weight, dtype=np.float32)
    assert x.shape == (T, H) and router_logits.shape == (T, E)
    assert w13_weight.shape == (E, TWO_I, H) and w2_weight.shape == (E, H, I)

    topw, rid, rtok, order, counts, offsets = _route(router_logits)
    n_chunks = max(1, -(-int(counts.max()) // CK))  # ceil; 1 in practice

    nc = _get_program()
    w13t_cores, w2t_cores = _prep_weights(w13_weight, w2_weight)

    # token rows per expert, in reference (stable) dispatch order
    expert_rows = [
        order[offsets[g]:offsets[g] + counts[g]] for g in range(E)
    ]

    ybuf = np.zeros((E, int(counts.max()), H), np.float32)

    for chunk in range(n_chunks):
        in_maps = []
        for c in range(N_CORES):
            xt_c = np.zeros((EPC, 128, KT1, CK), _np_wdt())
            for el in range(EPC):
                g = c * EPC + el
                rows = expert_rows[g][chunk * CK:(chunk + 1) * CK]
                if len(rows):
                    # [n, H] -> [H, n] -> [KT1, 128, n] -> [128, KT1, n]
                    xt_c[el, :, :, :len(rows)] = (
                        x[rtok[rows]].T.reshape(KT1, 128, -1).transpose(1, 0, 2)
                    )
            in_maps.append(
                {"w13t": w13t_cores[c], "w2t": w2t_cores[c], "xt": xt_c}
            )
        res = run_bass_kernel_spmd(nc, in_maps, list(range(N_CORES)))
        for c in range(N_CORES):
            yt_c = res.results[c]["yt"]  # [EPC, 2, 128, MT_G, CK]
            for el in range(EPC):
                g = c * EPC + el
                n = len(expert_rows[g][chunk * CK:(chunk + 1) * CK])
                if n:
                    lo = chunk * CK
                    # [MT_GRP, 128, MT_G, CK] -> y^T [H, CK] -> [n, H]
                    ytr = (
                        yt_c[el].transpose(0, 2, 1, 3).reshape(H, CK)
                    )
                    ybuf[g, lo:lo + n] = ytr[:, :n].T

    # ---- combine: gather rows back, weight by router probs ----
    pos = np.empty(T * TOP_K, np.int64)
    for g in range(E):
        pos[expert_rows[g]] = np.arange(counts[g])
    yrows = ybuf[rid, pos]  # [T*K, H]
    out = np.einsum(
        "tkh,tk->th", yrows.reshape(T, TOP_K, H), topw.astype(np.float32)
    )
    return out.astype(np.float32)
